# revision 9
# baseline (speedup 1.0000x reference)
"""Multi-head self-attention TRN2 Bass kernel, 8-way sharded.

Sharding: core c -> batch b = c//4, head-group hg = c%4 (4 heads each).
Per core: PE-transpose x_b -> xT (d-major); QT/KT d-major + V token-major
matmuls in bf16; flash attention in scores^T layout (softmax denominator via a
fused ones-column in the AV matmul lhsT; no max subtraction -- scores here are
bounded |s| < ~4); normalize with reciprocal_approx_fast + PE broadcast;
partial projection over the core's 256 ctx dims for all 2048 tokens.
Host sums the 4 per-batch partials and adds b_proj (the unshard step).
"""
import sys
import contextlib
sys.path.insert(0, '/opt/trn_rl_repo')
import numpy as np
import ml_dtypes

B, S, D = 2, 2048, 1024
H, HD = 16, 64
HPC = 4            # heads per core
CD = HPC * HD      # ctx dims per core = 256
NCORES = 8
NT = S // 128      # 16 token tiles
NK = D // 128      # 8 contraction tiles

_compiled = None


def _build():
    import concourse.bass as bass
    import concourse.bacc as bacc
    import concourse.tile as tile
    import concourse.mybir as mybir

    f32 = mybir.dt.float32
    bf16 = mybir.dt.bfloat16
    EXP = mybir.ActivationFunctionType.Exp

    nc = bacc.Bacc(None, num_devices=NCORES)
    x_d = nc.declare_dram_parameter("x", [S, D], bf16, False)
    wq_d = nc.declare_dram_parameter("wq", [D, CD], bf16, False)
    wk_d = nc.declare_dram_parameter("wk", [D, CD], bf16, False)
    wv_d = nc.declare_dram_parameter("wv", [D, CD], bf16, False)
    bq_d = nc.declare_dram_parameter("bq", [64, 4], f32, False)
    bk_d = nc.declare_dram_parameter("bk", [64, 4], f32, False)
    bvb_d = nc.declare_dram_parameter("bvb", [128, CD], f32, False)  # bcast
    wp_d = nc.declare_dram_parameter("wp", [CD, D], bf16, False)
    ident_d = nc.declare_dram_parameter("ident", [128, 128], bf16, False)
    shiftI_d = nc.declare_dram_parameter("shiftI", [128, 128], bf16, False)
    onesf_d = nc.declare_dram_parameter("onesf", [128, 128], f32, False)
    sel64_d = nc.declare_dram_parameter("sel64", [128, 128], f32, False)
    po_d = nc.declare_dram_parameter("po", [S, D], f32, True)  # partial out

    with tile.TileContext(nc) as tc:
        with contextlib.ExitStack() as ctx:
            # ---------------- persistent pools ----------------
            xt_pool = ctx.enter_context(tc.tile_pool(name="xt", bufs=1))
            qk_pool = ctx.enter_context(tc.tile_pool(name="qk", bufs=1))
            v_pool = ctx.enter_context(tc.tile_pool(name="vp", bufs=1))
            ctx_pool = ctx.enter_context(tc.tile_pool(name="ctx", bufs=1))
            const_pool = ctx.enter_context(tc.tile_pool(name="const", bufs=1))

            ident = const_pool.tile([128, 128], bf16, tag="ident")
            nc.sync.dma_start(ident[:], ident_d[:])
            bq_sb = const_pool.tile([64, 4], f32, tag="bq")
            bk_sb = const_pool.tile([64, 4], f32, tag="bk")
            nc.sync.dma_start(bq_sb[:], bq_d[:])
            nc.sync.dma_start(bk_sb[:], bk_d[:])
            bvb_sb = const_pool.tile([128, CD], f32, tag="bvb")
            nc.sync.dma_start(bvb_sb[:], bvb_d[:])

            # xT: 8 tiles [128 D, 2048 t] bf16
            xT = [xt_pool.tile([128, S], bf16, tag=f"xt{k}", name=f"xt{k}") for k in range(NK)]
            # QT/KT: 2 tiles each [128 d, 2048 t] bf16 (tile p: heads 2p,2p+1)
            QT = [qk_pool.tile([64, S], bf16, tag=f"qt{p}", name=f"qt{p}") for p in range(4)]
            KT = [qk_pool.tile([64, S], bf16, tag=f"kt{p}", name=f"kt{p}") for p in range(4)]
            # V': 16 tiles [128 t, 4*65] bf16 (head h cols 65h..65h+64 = V_h|1)
            VP = [v_pool.tile([128, HPC * (HD + 1)], bf16, tag=f"v{t}", name=f"v{t}")
                  for t in range(NT)]
            # ctxT: 2 tiles [128, 2048] bf16
            CTX = [ctx_pool.tile([128, S], bf16, tag=f"ctx{p}", name=f"ctx{p}") for p in range(2)]

            # ---------------- phase 0+1: transpose x, QKV ----------------
            with (
                tc.tile_pool(name="stage", bufs=8) as stage_pool,
                tc.tile_pool(name="w", bufs=1) as w_pool,
                tc.tile_pool(name="ps1", bufs=6, space="PSUM") as ps1,
            ):
                wq_sb = [w_pool.tile([128, CD], bf16, tag=f"wq{k}", name=f"wq{k}") for k in range(NK)]
                wk_sb = [w_pool.tile([128, CD], bf16, tag=f"wk{k}", name=f"wk{k}") for k in range(NK)]
                wv_sb = [w_pool.tile([128, CD], bf16, tag=f"wv{k}", name=f"wv{k}") for k in range(NK)]
                for kk in range(NK):
                    sl = slice(128 * kk, 128 * (kk + 1))
                    nc.sync.dma_start(wq_sb[kk][:], wq_d[sl, :])
                    nc.sync.dma_start(wk_sb[kk][:], wk_d[sl, :])
                    nc.sync.dma_start(wv_sb[kk][:], wv_d[sl, :])

                # transpose x in 4 column-bands of 4 t-tiles
                for tb in range(4):
                    stages = []
                    for q in range(4):
                        st = stage_pool.tile([128, D], bf16, tag="stage")
                        tt = 4 * tb + q
                        nc.sync.dma_start(st[:], x_d[128 * tt:128 * (tt + 1), :])
                        stages.append(st)
                    for kk in range(NK):
                        tp = ps1.tile([128, 512], bf16, tag="ps")
                        for q in range(4):
                            nc.tensor.transpose(
                                tp[:, 128 * q:128 * (q + 1)],
                                stages[q][:, 128 * kk:128 * (kk + 1)], ident[:])
                        nc.scalar.copy(xT[kk][:, 512 * tb:512 * (tb + 1)], tp[:])

                # QT/KT d-major per head: psum [64 d, 512 t], bias, cast bf16
                for h in range(4):
                    for (Wsb, bsb, DST) in ((wq_sb, bq_sb, QT), (wk_sb, bk_sb, KT)):
                        for t4 in range(4):
                            acc = ps1.tile([64, 512], f32, tag="ps")
                            for kk in range(NK):
                                nc.tensor.matmul(
                                    acc[:],
                                    Wsb[kk][:, 64 * h:64 * (h + 1)],
                                    xT[kk][:, 512 * t4:512 * (t4 + 1)],
                                    start=(kk == 0), stop=(kk == NK - 1))
                            nc.vector.tensor_scalar_add(
                                DST[h][:, 512 * t4:512 * (t4 + 1)], acc[:],
                                bsb[:, h:h + 1])

                # V token-major + bias, interleave ones cols
                for tt in range(NT):
                    acc = ps1.tile([128, CD], f32, tag="ps")
                    for kk in range(NK):
                        nc.tensor.matmul(
                            acc[:],
                            xT[kk][:, 128 * tt:128 * (tt + 1)],
                            wv_sb[kk][:],
                            start=(kk == 0), stop=(kk == NK - 1))
                    nc.vector.memset(VP[tt][:], 1.0)
                    nc.vector.tensor_add(
                        VP[tt][:].rearrange("p (h e) -> p h e", e=HD + 1)[:, :, 0:HD],
                        acc[:].rearrange("p (h e) -> p h e", e=HD),
                        bvb_sb[:].rearrange("p (h e) -> p h e", e=HD))

            # ---------------- phase 2: attention ----------------
            with (
                tc.tile_pool(name="sc", bufs=2, space="PSUM") as sc_pool,
                tc.tile_pool(name="av", bufs=2, space="PSUM") as av_pool,
                tc.tile_pool(name="e", bufs=3) as e_pool,
                tc.tile_pool(name="nrm", bufs=4) as nrm_pool,
                tc.tile_pool(name="ones", bufs=1) as ones_pool,
            ):
                onesf = ones_pool.tile([128, 128], f32, tag="onesf")
                nc.sync.dma_start(onesf[:], onesf_d[:])
                sel64 = ones_pool.tile([128, 128], f32, tag="sel64")
                nc.sync.dma_start(sel64[:], sel64_d[:])
                # shift identity: shiftI[k, m] = 1 iff m == k+64 (k<64)
                shiftI = ones_pool.tile([128, 128], bf16, tag="shiftI")
                nc.sync.dma_start(shiftI[:], shiftI_d[:])

                for j in range(4):          # q tiles of 512
                    qsl = slice(512 * j, 512 * (j + 1))
                    for p in range(2):      # head pairs
                        outp = [av_pool.tile([65, 512], f32, tag=f"av{hh}", name=f"av{hh}")
                                for hh in range(2)]
                        for i in range(NT):  # 16 key tiles
                            ksl = slice(128 * i, 128 * (i + 1))
                            sc = sc_pool.tile([128, 1024], f32, tag="sc")
                            for hh in range(2):
                                h = 2 * p + hh
                                nc.tensor.matmul(
                                    sc[:, 512 * hh:512 * (hh + 1)],
                                    KT[h][:, ksl],
                                    QT[h][:, qsl],
                                    start=True, stop=True)
                            ee = e_pool.tile([128, 1024], bf16, tag="e")
                            nc.scalar.activation(ee[:], sc[:], EXP, scale=0.125)
                            for hh in range(2):
                                h = 2 * p + hh
                                nc.tensor.matmul(
                                    outp[hh][:],
                                    VP[i][:, 65 * h:65 * h + 65],
                                    ee[:, 512 * hh:512 * (hh + 1)],
                                    start=(i == 0), stop=(i == NT - 1))
                        # normalize each head of the pair
                        for hh in range(2):
                            rsb = nrm_pool.tile([65, 512], f32, tag="rsb")
                            nc.vector.reciprocal_approx_fast(
                                rsb[:], outp[hh][:])
                            bc = sc_pool.tile([128, 1024], f32, tag="sc")
                            nc.tensor.matmul(
                                bc[0:64, 0:512],
                                sel64[0:65, 0:64],
                                rsb[:],
                                start=True, stop=True)
                            bcs = nrm_pool.tile([64, 512], f32, tag="bcs")
                            nc.vector.tensor_copy(bcs[:], bc[0:64, 0:512])
                            if hh == 0:
                                nc.vector.tensor_mul(
                                    CTX[p][0:64, qsl], outp[hh][0:64, :], bcs[:])
                            else:
                                tmp = nrm_pool.tile([64, 512], bf16, tag="tmp")
                                nc.vector.tensor_mul(
                                    tmp[:], outp[hh][0:64, :], bcs[:])
                                sh = sc_pool.tile([128, 1024], f32, tag="sc")
                                nc.tensor.matmul(
                                    sh[:, 0:512], shiftI[0:64, :], tmp[:],
                                    start=True, stop=True)
                                nc.vector.tensor_copy(
                                    CTX[p][64:128, qsl], sh[64:128, 0:512])

            # ---------------- phase 3: partial projection ----------------
            with (
                tc.tile_pool(name="wp", bufs=1) as wp_pool,
                tc.tile_pool(name="po", bufs=3) as po_pool,
                tc.tile_pool(name="ps3", bufs=4, space="PSUM") as ps3,
            ):
                wp_sb = [wp_pool.tile([128, D], bf16, tag=f"wp{k}", name=f"wp{k}") for k in range(2)]
                for kk in range(2):
                    nc.sync.dma_start(wp_sb[kk][:], wp_d[128 * kk:128 * (kk + 1), :])
                for tt in range(NT):
                    tsl = slice(128 * tt, 128 * (tt + 1))
                    for nn in range(2):
                        nsl = slice(512 * nn, 512 * (nn + 1))
                        acc = ps3.tile([128, 512], f32, tag="ps")
                        for kk in range(2):
                            nc.tensor.matmul(
                                acc[:], CTX[kk][:, tsl], wp_sb[kk][:, nsl],
                                start=(kk == 0), stop=(kk == 1))
                        ot = po_pool.tile([128, 512], f32, tag="po")
                        nc.vector.tensor_copy(ot[:], acc[:])
                        nc.sync.dma_start(po_d[tsl, nsl], ot[:])
    nc.compile()
    return nc


def kernel(x, W_qkv, b_qkv, W_proj, b_proj):
    global _compiled
    from concourse.bass_utils import run_bass_kernel_spmd

    x = np.asarray(x, dtype=np.float32)
    W_qkv = np.asarray(W_qkv, dtype=np.float32)
    b_qkv = np.asarray(b_qkv, dtype=np.float32)
    W_proj = np.asarray(W_proj, dtype=np.float32)
    b_proj = np.asarray(b_proj, dtype=np.float32)

    if _compiled is None:
        _compiled = _build()
    nc = _compiled

    bf = ml_dtypes.bfloat16
    ident_np = np.eye(128, dtype=bf)
    shiftI_np = np.zeros((128, 128), dtype=np.float32)
    shiftI_np[np.arange(64), np.arange(64) + 64] = 1.0
    shiftI_np = shiftI_np.astype(bf)
    sel64_np = np.zeros((128, 128), dtype=np.float32)
    sel64_np[64, :] = 1.0
    in_maps = []
    for c in range(NCORES):
        b, hg = c // 4, c % 4
        cs = slice(CD * hg, CD * (hg + 1))
        in_maps.append({
            "x": x[b].astype(bf),
            "wq": np.ascontiguousarray(W_qkv[:, 0:D][:, cs]).astype(bf),
            "wk": np.ascontiguousarray(W_qkv[:, D:2 * D][:, cs]).astype(bf),
            "wv": np.ascontiguousarray(W_qkv[:, 2 * D:3 * D][:, cs]).astype(bf),
            "bq": np.ascontiguousarray(b_qkv[0:D][cs].reshape(4, 64).T),
            "bk": np.ascontiguousarray(b_qkv[D:2 * D][cs].reshape(4, 64).T),
            "bvb": np.tile(b_qkv[2 * D:3 * D][cs], (128, 1)).astype(np.float32),
            "wp": np.ascontiguousarray(W_proj[cs, :]).astype(bf),
            "ident": ident_np,
            "shiftI": shiftI_np,
            "onesf": np.ones((128, 128), dtype=np.float32),
            "sel64": sel64_np,
        })

    res = run_bass_kernel_spmd(nc, in_maps, list(range(NCORES)))
    out = np.zeros((B, S, D), dtype=np.float32)
    for b in range(B):
        acc = np.zeros((S, D), dtype=np.float32)
        for hg in range(4):
            acc += res.results[4 * b + hg]["po"]
        out[b] = acc + b_proj
    return out
